# revision 65
# baseline (speedup 1.0000x reference)
"""Trainium2 Bass kernel for nn_Encoder_P: unwrap-diff-square front-end + 4 dilated
convs with dense concatenation, fused end-to-end on-chip.

Strategy (pure data parallel, 1 batch sample per NeuronCore, 8 cores):
  - The unwrap/diff/pad chain collapses: cumsum cancels in the diff, so
    sq[h] = wrap(p[h] - p[h-1])^2 (row 0 = 0), wrap(v) = v - 2*pi*k with
    k = (v>=pi) + (v>=3pi) - (v<=-pi) - (v<=-3pi).
  - Duplicate concat channels are folded into effective conv weights
    (conv3: 8->7 input planes, conv4: 20->15).
  - Each conv runs on TensorE as banded matmuls over the H (partition) axis:
    lhsT is a banded [128,128] H-shift matrix; rhs is the input plane tile
    [128 H, 516 Wpad]; PSUM accumulates over (ci, kw). Planes are stored as
    5 overlapping H-tiles (stride 107, halo 12) with zeroed W margins.

Production config (KCFG):
  - bf16 planes + bf16 matmuls; conv1-3 band tables precomputed on host and
    DMA'd in (no per-band DVE build); bf16 DRAM output, upcast on host.
  - conv4 (1x1, 15 planes -> 16 ch) runs as fp8e4m3 DoubleRow matmuls: the
    14 small-magnitude planes are DVE-converted into 7 halo-grid pair
    tensors ([K,2,N] APs, K-doubled contraction, ~0.7x stream cycles per
    the HW DoubleRow rate); sq (magnitude ~pi^2) stays bf16 via one extra
    plain matmul per (o, tile). Matmul stream count: 1980 -> 1420, with
    560 of those at DoubleRow rate.
  - conv3 (2x2 dil3, 7 planes -> 8 ch) likewise: c2/c1 pairs via fp8
    DoubleRow (bands host-built), sq term bf16. 1180 total matmul streams
    (800 at DoubleRow rate) vs 1980 in the f32r baseline.
  - Output channels 16-47 stream early (right after their source plane is
    final); duplicated channels are written once from SBUF then replicated
    DRAM->DRAM (1 dma_start each); conv3/conv4 stores batch tiles 0-3 into
    a single 3D-AP dma_start — HWDGE descriptor-gen serial time drops from
    ~224us (naive) to ~72us, well under the TensorE stream time.
"""

import numpy as np

import concourse.bacc as bacc
import concourse.bass as bass
import concourse.mybir as mybir
import concourse.tile as tile
from concourse import bass_utils

F32 = mybir.dt.float32
MM_DT = mybir.dt.float32r  # full-rate fp32 matmul path (1 cyc/row at N>=256)
DEFAULT_MM = "f32r"  # flip to "bf16" only with HW-validated accuracy+speed

H = 512
W = 512
S = 107          # tile stride in rows (chosen so 512-(S*4-HALO) == 96, a legal
                 # compute-op partition start for the bottom edge-zero memset)
HALO = 12        # halo rows above/below each tile
NT = 5           # number of H tiles
WPAD = 516       # 2 zero cols + 512 + 2 zero cols
WA = 528         # fp8 pair-slot stride (16B-aligned for DoubleRow APs)
P = 128
PI = float(np.pi)

# conv specs: (dil, pad_top, pad_left, KH, KW)
CONV_GEOM = [
    (1, 1, 1, 4, 4),   # conv1: 4x4 dil1, 'same' pad (1,2)
    (2, 2, 2, 3, 3),   # conv2: 3x3 dil2, pad (2,2)
    (3, 1, 1, 2, 2),   # conv3: 2x2 dil3, pad (1,2)
    (4, 0, 0, 1, 1),   # conv4: 1x1
]

PLANE_NAMES = (
    ["sq", "c1_0", "c1_1"]
    + [f"c2_{i}" for i in range(4)]
    + [f"c3_{i}" for i in range(8)]
)
CONV_INPUTS = [
    ["sq"],
    ["c1_0", "c1_1", "sq"],
    [f"c2_{i}" for i in range(4)] + ["c1_0", "c1_1", "sq"],
    [f"c3_{i}" for i in range(8)] + [f"c2_{i}" for i in range(4)]
    + ["c1_0", "c1_1", "sq"],
]
CONV_OUT = [2, 4, 8, 16]
DELTAS = [-2, -1, 0, 1, 2]  # identity master shifts

# output channel -> source plane ("c4_o" channels handled separately)
CH_MAP = (
    [f"c4_{i}" for i in range(16)]
    + [f"c3_{i}" for i in range(8)]
    + [f"c2_{i}" for i in range(4)]
    + ["c1_0", "c1_1", "sq", "sq", "c1_0", "c1_1", "sq", "sq"]
    + [f"c2_{i}" for i in range(4)]
    + ["c1_0", "c1_1", "sq", "sq"]
    + ["c1_0", "c1_1", "sq", "sq"]
)

NSCAL = sum(
    CONV_OUT[c] * len(CONV_INPUTS[c]) * CONV_GEOM[c][3] * CONV_GEOM[c][4]
    for c in range(4)
)  # 604


def _fold_weights(w1, w2, w3, w4):
    w3f = np.zeros((8, 7, 2, 2), np.float32)
    w3f[:, :6] = w3[:, :6]
    w3f[:, 6] = w3[:, 6] + w3[:, 7]
    w4f = np.zeros((16, 15, 1, 1), np.float32)
    w4f[:, :12] = w4[:, :12]
    w4f[:, 12] = w4[:, 12] + w4[:, 16]
    w4f[:, 13] = w4[:, 13] + w4[:, 17]
    w4f[:, 14] = w4[:, 14] + w4[:, 15] + w4[:, 18] + w4[:, 19]
    return [w1.astype(np.float32), w2.astype(np.float32), w3f, w4f]


N_BANDS_DMA = 8 + 36 + 112  # conv1-3 (o,ci,kw) bands; conv4 diags built on DVE


def _host_tables(inputs):
    """wtab [128, NSCAL], ident [5*128, 128], bias [128, 30], bands
    [128, (N_BANDS_DMA+1)*128] bf16 host arrays (last slot = identity)."""
    import ml_dtypes

    wf = _fold_weights(inputs["w1"], inputs["w2"], inputs["w3"], inputs["w4"])
    scal = []
    for c in range(4):
        dil, pad_top, _, KH, KW = CONV_GEOM[c]
        for o in range(CONV_OUT[c]):
            for ci in range(len(CONV_INPUTS[c])):
                for kw in range(KW):
                    for kh in range(KH):
                        scal.append(wf[c][o, ci, kh, kw])
    assert len(scal) == NSCAL
    wtab = np.tile(np.asarray(scal, np.float32)[None, :], (P, 1))
    ident = np.concatenate(
        [np.eye(P, dtype=np.float32, k=-d) for d in DELTAS], axis=0
    )
    bias = np.concatenate(
        [inputs["b1"], inputs["b2"], inputs["b3"], inputs["b4"]]
    ).astype(np.float32)
    bias = np.tile(bias[None, :], (P, 1))

    # band tables for conv1-3, partition-major [128, nb*128]
    # trailing slots: identity, half-identity (for the (sq,sq) self-pair)
    bands = np.zeros((P, (N_BANDS_DMA + 2) * P), np.float32)
    j = 0
    for c in range(3):
        dil, pad_top, _, KH, KW = CONV_GEOM[c]
        deltas = [kh * dil - pad_top for kh in range(KH)]
        for o in range(CONV_OUT[c]):
            for ci in range(len(CONV_INPUTS[c])):
                for kw in range(KW):
                    b = np.zeros((P, P), np.float32)
                    for kh, d in enumerate(deltas):
                        b += wf[c][o, ci, kh, kw] * np.eye(P, k=-d)
                    bands[:, j * P : (j + 1) * P] = b
                    j += 1
    assert j == N_BANDS_DMA
    bands[:, j * P : (j + 1) * P] = np.eye(P)
    bands[:, (j + 1) * P : (j + 2) * P] = 0.5 * np.eye(P)
    bands = bands.astype(ml_dtypes.bfloat16)

    # fp8 DoubleRow pair-bands for conv3: (o, pair, kw, slot) -> banded
    ci3 = {"c2_0": 0, "c2_1": 1, "c2_2": 2, "c2_3": 3, "c1_0": 4, "c1_1": 5}
    d3 = [kh * 3 - 1 for kh in range(2)]  # conv3 deltas
    c3b = np.zeros((P, 8 * 3 * 2 * 2 * P), np.float32)
    for o in range(8):
        for pj, pr in enumerate(PAIRS[4:7]):
            for kw in range(2):
                for i in (0, 1):
                    b = np.zeros((P, P), np.float32)
                    for kh, d in enumerate(d3):
                        b += wf[2][o, ci3[pr[i]], kh, kw] * np.eye(P, k=-d)
                    k = (((o * 3 + pj) * 2 + kw) * 2 + i) * P
                    c3b[:, k : k + P] = b
    c3b = c3b.astype(ml_dtypes.float8_e4m3)
    return wtab, ident, bias, bands, c3b


# conv4 DoubleRow plane pairs; sq stays bf16 (its magnitude ~pi^2 makes
# fp8 quantization the dominant output error)
PAIRS = [
    ("c3_0", "c3_1"), ("c3_2", "c3_3"), ("c3_4", "c3_5"), ("c3_6", "c3_7"),
    ("c2_0", "c2_1"), ("c2_2", "c2_3"), ("c1_0", "c1_1"),
]
# plane name -> column in folded w4f
W4COL = {f"c3_{i}": i for i in range(8)}
W4COL.update({f"c2_{i}": 8 + i for i in range(4)})
W4COL.update({"c1_0": 12, "c1_1": 13, "sq": 14})
PAIR_OF = {}
for _j, (_a, _b) in enumerate(PAIRS):
    PAIR_OF.setdefault(_a, []).append((_j, 0))
    PAIR_OF.setdefault(_b, []).append((_j, 1))


def build_nc(loop_k=1, out_mode='full', skip_bands=False, mm='f32r',
             col_split=1, n_convs=4, band_src='dve', out_dt='f32',
             conv4_exact=False, dr4=False, dr3=False):
    assert band_src in ('dve', 'dma')
    if band_src == 'dma':
        assert mm == 'bf16', "dma band tables are bf16"
    if conv4_exact:
        assert mm == 'bf16' and out_dt == 'bf16' and n_convs == 4
    if dr4:
        assert band_src == 'dma' and out_dt == 'bf16' and n_convs == 4
        assert not conv4_exact
    if dr3:
        assert dr4, "dr3 reuses dr4's fp8 pair tensors"
    nc = bacc.Bacc("TRN2", target_bir_lowering=False, debug=False)
    mm_dt = mybir.dt.bfloat16 if mm == 'bf16' else MM_DT
    o_dt = mybir.dt.bfloat16 if out_dt == 'bf16' else F32

    def msafe(ap):
        # memset target: walrus rejects float32r memsets; bitcast those to f32
        return ap.bitcast(F32) if mm != 'bf16' else ap

    p_dram = nc.dram_tensor("p", [H, W], F32, kind="ExternalInput")
    ident_dram = nc.dram_tensor("ident", [5 * P, P], F32, kind="ExternalInput")
    wtab_dram = nc.dram_tensor("wtab", [P, NSCAL], F32, kind="ExternalInput")
    bias_dram = nc.dram_tensor("bias", [P, 30], F32, kind="ExternalInput")
    bands_dram = (
        nc.dram_tensor(
            "bands", [P, (N_BANDS_DMA + 2) * P], mybir.dt.bfloat16,
            kind="ExternalInput",
        )
        if band_src == 'dma'
        else None
    )
    out_dram = nc.dram_tensor("out", [48, H, W], o_dt, kind="ExternalOutput")

    FP8 = mybir.dt.float8e4
    # halo-grid planes; with conv4_exact/dr4, c3 skips the halo layout
    halo_names = [
        nm for nm in PLANE_NAMES
        if not ((conv4_exact or dr4) and nm.startswith("c3_"))
    ]
    planes = {
        nm: nc.alloc_sbuf_tensor(f"pl_{nm}", [P, NT * WPAD], mm_dt)
        for nm in halo_names
    }
    # exact-128-row tiles (4 per plane, no W margins) for conv4 inputs
    xnames = CONV_INPUTS[3] if conv4_exact else []
    xplanes = {
        nm: nc.alloc_sbuf_tensor(f"xp_{nm}", [P, 4 * W], mm_dt)
        for nm in xnames
    }
    # fp8 halo-grid pair-planes for DoubleRow conv4: dim1 = (tile, slot),
    # dim2 = WA cols (16B-aligned pair stride; data at [2, 514))
    fp8_pairs = (
        [
            nc.alloc_sbuf_tensor(f"f8p_{j}", [P, NT * 2, WA], FP8)
            for j in range(len(PAIRS))
        ]
        if dr4
        else None
    )
    # DoubleRow pair-band table: (o, j) -> [128, 2, 128] diag pair
    c4b_sb = (
        nc.alloc_sbuf_tensor("c4b_sb", [P, 16 * len(PAIRS) * 2 * P], FP8)
        if dr4
        else None
    )
    # bf16 diag bands for conv4's sq term
    sqb_sb = (
        nc.alloc_sbuf_tensor("sqb_sb", [P, 16 * P], mybir.dt.bfloat16)
        if dr4
        else None
    )
    # fp8 DoubleRow pair-bands for conv3 (host-built, DMA'd)
    c3b_dram = (
        nc.dram_tensor("c3b", [P, 96 * P], FP8, kind="ExternalInput")
        if dr3
        else None
    )
    c3b_sb = (
        nc.alloc_sbuf_tensor("c3b_sb", [P, 96 * P], FP8) if dr3 else None
    )
    ident_sb = nc.alloc_sbuf_tensor("ident_sb", [P, 5 * P], F32)
    wtab_sb = nc.alloc_sbuf_tensor("wtab_sb", [P, NSCAL], F32)
    bias_sb = nc.alloc_sbuf_tensor("bias_sb", [P, 30], F32)
    bands_sb = (
        nc.alloc_sbuf_tensor(
            "bands_sb", [P, (N_BANDS_DMA + 2) * P], mybir.dt.bfloat16
        )
        if band_src == 'dma'
        else None
    )

    def pslice(nm, t, c0, c1):
        return planes[nm][:, t * WPAD + c0 : t * WPAD + c1]

    with tile.TileContext(nc) as tc:
        with (
            tc.tile_pool(name="io", bufs=3) as io_pool,
            tc.tile_pool(name="front", bufs=1 if (dr3 or dr4) else 2) as fr_pool,
            tc.tile_pool(name="bands", bufs=12) as band_pool,
            tc.tile_pool(name="psum", bufs=8, space="PSUM") as psum_pool,
            tc.tile_pool(name="c4st", bufs=3) as c4_pool,
            tc.tile_pool(name="f8", bufs=3) as f8_pool,
        ):
            for _it in range(loop_k):
                # ---- parameter loads ----
                if band_src == 'dve' or skip_bands:
                    for j in range(5):
                        nc.sync.dma_start(
                            out=ident_sb[:, j * P : (j + 1) * P],
                            in_=ident_dram[j * P : (j + 1) * P, :],
                        )
                if band_src == 'dma':
                    nc.sync.dma_start(out=bands_sb[:], in_=bands_dram[:])
                if dr3:
                    nc.sync.dma_start(out=c3b_sb[:], in_=c3b_dram[:])
                    # zero pair tensors so W margins read as reference padding
                    for j_ in range(len(PAIRS)):
                        for s_ in range(NT * 2):
                            nc.gpsimd.memset(
                                fp8_pairs[j_][:, s_, :].bitcast(F32), 0.0
                            )
                nc.sync.dma_start(out=wtab_sb[:], in_=wtab_dram[:])
                nc.sync.dma_start(out=bias_sb[:], in_=bias_dram[:])

                # ---- zero W margins of all planes (written once) ----
                for nm in halo_names:
                    for t in range(NT):
                        nc.gpsimd.memset(msafe(pslice(nm, t, 0, 2)), 0.0)
                        nc.gpsimd.memset(msafe(pslice(nm, t, 514, 516)), 0.0)

                def emit_chans(nm, dedup=False):
                    """DMA output channels sourced from plane nm. With dedup,
                    write the first channel from SBUF and replicate the rest
                    via DRAM->DRAM copies (1 dma_start each vs NT)."""
                    if out_mode != 'full':
                        return
                    chans = [ch for ch in range(16, 48) if CH_MAP[ch] == nm]
                    for ci_, ch in enumerate(chans):
                        if dedup and ci_ > 0:
                            nc.sync.dma_start(
                                out=out_dram[ch, :, :],
                                in_=out_dram[chans[0], :, :],
                            )
                            continue
                        for t in range(NT):
                            rows = S if t < NT - 1 else H - S * (NT - 1)
                            nc.sync.dma_start(
                                out=out_dram[ch, S * t : S * t + rows, :],
                                in_=planes[nm][
                                    HALO : HALO + rows,
                                    t * WPAD + 2 : t * WPAD + 514,
                                ],
                            )

                def to_exact(nm, t, rows, src_tile):
                    """DMA rows [S*t, S*t+rows) of src_tile (halo-grid tile:
                    partition HALO+r = image row S*t+r, 512-wide cols) into
                    xplanes[nm] exact tiles."""
                    r0 = S * t
                    r1 = r0 + rows
                    e0, e1 = r0 // 128, (r1 - 1) // 128
                    for e in range(e0, e1 + 1):
                        lo = max(r0, 128 * e)
                        hi = min(r1, 128 * (e + 1))
                        nc.sync.dma_start(
                            out=xplanes[nm][lo - 128 * e : hi - 128 * e,
                                            e * W : e * W + W],
                            in_=src_tile[HALO + lo - r0 : HALO + hi - r0, :],
                        )

                def to_pair(nm, t, src_ap):
                    """DVE-convert a halo-grid [128, 512] bf16 AP into the fp8
                    pair slot(s) of plane nm at tile t (same partitions)."""
                    for (pj, pi) in PAIR_OF.get(nm, []):
                        nc.vector.tensor_scalar(
                            fp8_pairs[pj][:, 2 * t + pi, 2:514],
                            src_ap, 1.0, None, mybir.AluOpType.mult,
                        )

                def conv_to_pairs(nm):
                    for t in range(NT):
                        to_pair(
                            nm, t, planes[nm][:, t * WPAD + 2 : t * WPAD + 514]
                        )

                # ---- front-end: sq ----
                # A/B garbage regions are pre-zeroed so the out-of-image rows
                # compute v=0 -> sq=0, which is exactly the reference's zero pad.
                for t in range(NT):
                    p_lo = HALO if t == 0 else 0
                    p_hi = H - (S * (NT - 1) - HALO) if t == NT - 1 else P  # 96 at t=4
                    n = p_hi - p_lo
                    r_lo = S * t - HALO + p_lo
                    A = io_pool.tile([P, W], F32, tag="A")
                    B = io_pool.tile([P, W], F32, tag="B")
                    if t == 0:
                        nc.gpsimd.memset(A[0:32, :], 0.0)
                        nc.gpsimd.memset(B[0:32, :], 0.0)
                    if t == NT - 1:
                        nc.gpsimd.memset(A[96:P, :], 0.0)
                        nc.gpsimd.memset(B[96:P, :], 0.0)
                    nc.sync.dma_start(out=A[p_lo:p_hi, :], in_=p_dram[r_lo : r_lo + n, :])
                    if t == 0:
                        nc.sync.dma_start(
                            out=B[p_lo + 1 : p_hi, :], in_=p_dram[0 : n - 1, :]
                        )
                        nc.sync.dma_start(out=B[p_lo : p_lo + 1, :], in_=p_dram[0:1, :])
                    else:
                        nc.sync.dma_start(
                            out=B[p_lo:p_hi, :], in_=p_dram[r_lo - 1 : r_lo - 1 + n, :]
                        )
                    V = fr_pool.tile([P, W], F32, tag="V")
                    K1 = fr_pool.tile([P, W], F32, tag="K1")
                    K2 = fr_pool.tile([P, W], F32, tag="K2")
                    K3 = fr_pool.tile([P, W], F32, tag="K3")
                    K4 = fr_pool.tile([P, W], F32, tag="K4")
                    ao = mybir.AluOpType
                    nc.vector.tensor_tensor(V[:], A[:], B[:], ao.subtract)
                    nc.vector.tensor_scalar(K1[:], V[:], PI, None, ao.is_ge)
                    nc.vector.tensor_scalar(K2[:], V[:], 3 * PI, None, ao.is_ge)
                    nc.vector.tensor_scalar(K3[:], V[:], -PI, None, ao.is_le)
                    nc.vector.tensor_scalar(K4[:], V[:], -3 * PI, None, ao.is_le)
                    nc.vector.tensor_tensor(K1[:], K1[:], K2[:], ao.add)
                    nc.vector.tensor_tensor(K3[:], K3[:], K4[:], ao.add)
                    nc.vector.tensor_tensor(K1[:], K1[:], K3[:], ao.subtract)
                    nc.vector.scalar_tensor_tensor(
                        V[:], K1[:], -2 * PI, V[:], ao.mult, ao.add
                    )
                    sq_dst = planes["sq"][:, t * WPAD + 2 : t * WPAD + 514]
                    nc.vector.tensor_tensor(sq_dst, V[:], V[:], ao.mult)

                if conv4_exact or dr4:
                    emit_chans("sq", dedup=dr4)
                    if conv4_exact:
                        for t in range(NT):
                            rows = S if t < NT - 1 else H - S * (NT - 1)
                            to_exact(
                                "sq", t, rows,
                                planes["sq"][:, t * WPAD + 2 : t * WPAD + 514],
                            )

                if dr4:
                    # DoubleRow pair-band table: diag(w) pairs, built on DVE
                    base4 = NSCAL - 240
                    NPJ = len(PAIRS)
                    ident_ap = bands_sb[
                        :, N_BANDS_DMA * P : (N_BANDS_DMA + 1) * P
                    ]
                    for o4 in range(16):
                        for pj, pr in enumerate(PAIRS):
                            for pi in (0, 1):
                                col = base4 + o4 * 15 + W4COL[pr[pi]]
                                k = ((o4 * NPJ + pj) * 2 + pi) * P
                                nc.vector.tensor_scalar(
                                    c4b_sb[:, k : k + P],
                                    ident_ap,
                                    wtab_sb[:, col : col + 1],
                                    None,
                                    mybir.AluOpType.mult,
                                )
                        colq = base4 + o4 * 15 + 14
                        nc.vector.tensor_scalar(
                            sqb_sb[:, o4 * P : (o4 + 1) * P],
                            ident_ap,
                            wtab_sb[:, colq : colq + 1],
                            None,
                            mybir.AluOpType.mult,
                        )

                # ---- convs ----
                jcol = 0
                bias_col = 0
                bidx = 0
                p_hi_last = H - (S * (NT - 1) - HALO)  # 108
                for c in range(n_convs):
                    dil, pad_top, pad_left, KH, KW = CONV_GEOM[c]
                    in_names = CONV_INPUTS[c]
                    O = CONV_OUT[c]
                    deltas = [kh * dil - pad_top for kh in range(KH)]
                    if c == 2 and dr3:
                        DR = mybir.MatmulPerfMode.DoubleRow
                        c3base = 8 + 36
                        for o in range(O):
                            psums = [
                                psum_pool.tile(
                                    [P, W], F32, tag="ps", name=f"ps3_{o}_{t}"
                                )
                                for t in range(NT)
                            ]
                            for pj3 in range(3):
                                for kw in range(2):
                                    k = (((o * 3 + pj3) * 2 + kw) * 2) * P
                                    lhsT = c3b_sb[:, k : k + 2 * P].rearrange(
                                        "p (two m) -> p two m", two=2
                                    )
                                    coff = 2 + kw * 3 - 1
                                    for t in range(NT):
                                        rhs = fp8_pairs[4 + pj3][
                                            :, 2 * t : 2 * t + 2,
                                            coff : coff + W,
                                        ]
                                        nc.tensor.matmul(
                                            psums[t], lhsT, rhs,
                                            start=(pj3 == 0 and kw == 0),
                                            stop=False,
                                            perf_mode=DR,
                                        )
                            for kw in range(2):
                                bidx3 = c3base + (o * 7 + 6) * 2 + kw
                                band = bands_sb[
                                    :, bidx3 * P : (bidx3 + 1) * P
                                ]
                                coff = 2 + kw * 3 - 1
                                for t in range(NT):
                                    nc.tensor.matmul(
                                        psums[t],
                                        band,
                                        planes["sq"][
                                            :,
                                            t * WPAD + coff : t * WPAD
                                            + coff + W,
                                        ],
                                        start=False,
                                        stop=(kw == 1),
                                    )
                            bias_ap = bias_sb[:, bias_col + o : bias_col + o + 1]
                            st4 = c4_pool.tile([P, 4 * W], o_dt, tag="c4w")
                            for t in range(4):
                                nc.scalar.add(
                                    st4[:, t * W : (t + 1) * W],
                                    psums[t][:], bias_ap,
                                )
                                to_pair(
                                    f"c3_{o}", t, st4[:, t * W : (t + 1) * W]
                                )
                            nc.sync.dma_start(
                                out=out_dram[16 + o, 0 : 4 * S, :].rearrange(
                                    "(t r) w -> r t w", t=4
                                ),
                                in_=st4[HALO : HALO + S, :].rearrange(
                                    "p (t w) -> p t w", t=4
                                ),
                            )
                            st = c4_pool.tile([P, W], o_dt, tag="c4")
                            nc.scalar.add(st[:], psums[4][:], bias_ap)
                            nc.sync.dma_start(
                                out=out_dram[16 + o, 4 * S : H, :],
                                in_=st[HALO : HALO + H - 4 * S, :],
                            )
                            to_pair(f"c3_{o}", 4, st[:])
                        bias_col += O
                        jcol += 224
                        continue

                    if c == 3 and dr4:
                        DR = mybir.MatmulPerfMode.DoubleRow
                        for o in range(O):
                            psums = [
                                psum_pool.tile(
                                    [P, W], F32, tag="ps", name=f"ps4_{o}_{t}"
                                )
                                for t in range(NT)
                            ]
                            for pj in range(len(PAIRS)):
                                k = (o * len(PAIRS) + pj) * 2 * P
                                lhsT = c4b_sb[:, k : k + 2 * P].rearrange(
                                    "p (two m) -> p two m", two=2
                                )
                                for t in range(NT):
                                    rhs = fp8_pairs[pj][:, 2 * t : 2 * t + 2, 2:514]
                                    nc.tensor.matmul(
                                        psums[t], lhsT, rhs,
                                        start=(pj == 0),
                                        stop=False,
                                        perf_mode=DR,
                                    )
                            for t in range(NT):
                                nc.tensor.matmul(
                                    psums[t],
                                    sqb_sb[:, o * P : (o + 1) * P],
                                    planes["sq"][
                                        :, t * WPAD + 2 : t * WPAD + 514
                                    ],
                                    start=False,
                                    stop=True,
                                )
                            bias_ap = bias_sb[:, bias_col + o : bias_col + o + 1]
                            st4 = c4_pool.tile([P, 4 * W], o_dt, tag="c4w")
                            for t in range(4):
                                nc.scalar.add(
                                    st4[:, t * W : (t + 1) * W],
                                    psums[t][:], bias_ap,
                                )
                            nc.sync.dma_start(
                                out=out_dram[o, 0 : 4 * S, :].rearrange(
                                    "(t r) w -> r t w", t=4
                                ),
                                in_=st4[HALO : HALO + S, :].rearrange(
                                    "p (t w) -> p t w", t=4
                                ),
                            )
                            st = c4_pool.tile([P, W], o_dt, tag="c4")
                            nc.scalar.add(st[:], psums[4][:], bias_ap)
                            nc.sync.dma_start(
                                out=out_dram[o, 4 * S : H, :],
                                in_=st[HALO : HALO + H - 4 * S, :],
                            )
                        jcol += 240
                        bias_col += O
                        continue
                    x4 = conv4_exact and c == 3
                    NTC = 4 if x4 else NT
                    for o in range(O):
                        psums = [
                            psum_pool.tile([P, W], F32, tag="ps", name=f"ps_{c}_{o}_{t}")
                            for t in range(NTC)
                        ]
                        for ci, nm in enumerate(in_names):
                            for kw in range(KW):
                                use_dma_band = band_src == 'dma' and c < 3
                                if skip_bands:
                                    band = None
                                    jcol += len(deltas)
                                    if use_dma_band:
                                        bidx += 1
                                elif use_dma_band:
                                    band = bands_sb[:, bidx * P : (bidx + 1) * P]
                                    bidx += 1
                                    jcol += len(deltas)
                                elif band_src == 'dma':
                                    # conv4: diagonal band = w * I, one DVE op
                                    band = band_pool.tile([P, P], mm_dt, tag="band")
                                    w_ap = wtab_sb[:, jcol : jcol + 1]
                                    jcol += len(deltas)
                                    nc.vector.tensor_scalar(
                                        band[:],
                                        bands_sb[
                                            :, N_BANDS_DMA * P : (N_BANDS_DMA + 1) * P
                                        ],
                                        w_ap,
                                        None,
                                        mybir.AluOpType.mult,
                                    )
                                else:
                                    band = band_pool.tile([P, P], mm_dt, tag="band")
                                    for i, d in enumerate(deltas):
                                        w_ap = wtab_sb[:, jcol : jcol + 1]
                                        jcol += 1
                                        src = ident_sb[
                                            :, (d + 2) * P : (d + 3) * P
                                        ]
                                        ao = mybir.AluOpType
                                        if i == 0:
                                            nc.vector.tensor_scalar(
                                                band[:], src, w_ap, None, ao.mult
                                            )
                                        else:
                                            nc.vector.scalar_tensor_tensor(
                                                band[:], src, w_ap, band[:],
                                                ao.mult, ao.add
                                            )
                                coff = 2 + kw * dil - pad_left
                                first = ci == 0 and kw == 0
                                last = ci == len(in_names) - 1 and kw == KW - 1
                                for t in range(NTC):
                                    rhs = (
                                        xplanes[nm][:, t * W : (t + 1) * W]
                                        if x4
                                        else planes[nm][
                                            :, t * WPAD + coff : t * WPAD + coff + W
                                        ]
                                    )
                                    if skip_bands:
                                        lhsT = (
                                            ident_sb[:, 2 * P : 3 * P].bitcast(mm_dt)
                                            if mm != "bf16"
                                            else ident_sb[:, 2 * P : 3 * P]
                                        )
                                    elif use_dma_band:
                                        lhsT = band
                                    else:
                                        lhsT = band[:]
                                    if col_split == 1:
                                        nc.tensor.matmul(
                                            psums[t], lhsT, rhs,
                                            start=first, stop=last,
                                        )
                                    else:
                                        cw = P // col_split
                                        for j in range(col_split):
                                            nc.tensor.matmul(
                                                psums[t][j * cw : (j + 1) * cw, :],
                                                lhsT[:, j * cw : (j + 1) * cw],
                                                rhs,
                                                start=first,
                                                stop=last,
                                                tile_position=(0, j * cw),
                                                skip_group_check=True,
                                            )
                        bias_ap = bias_sb[:, bias_col + o : bias_col + o + 1]
                        if c == 2 and (conv4_exact or dr4):
                            # conv3: evac to scratch; c3 out channel + exact repack
                            for t in range(NT):
                                st = c4_pool.tile([P, W], o_dt, tag="c4")
                                nc.scalar.add(st[:], psums[t][:], bias_ap)
                                rows = S if t < NT - 1 else H - S * (NT - 1)
                                nc.sync.dma_start(
                                    out=out_dram[16 + o, S * t : S * t + rows, :],
                                    in_=st[HALO : HALO + rows, :],
                                )
                                if conv4_exact:
                                    to_exact(f"c3_{o}", t, rows, st[:, :])
                                else:
                                    to_pair(f"c3_{o}", t, st[:])
                        elif c < 3:
                            out_nm = (
                                ["c1_0", "c1_1"][o]
                                if c == 0
                                else (f"c2_{o}" if c == 1 else f"c3_{o}")
                            )
                            for t in range(NT):
                                nc.scalar.add(
                                    pslice(out_nm, t, 2, 514), psums[t][:], bias_ap
                                )
                        elif x4:
                            for e in range(4):
                                st = c4_pool.tile([P, W], o_dt, tag="c4")
                                nc.scalar.add(st[:], psums[e][:], bias_ap)
                                nc.sync.dma_start(
                                    out=out_dram[o, 128 * e : 128 * (e + 1), :],
                                    in_=st[:],
                                )
                        else:
                            for t in range(NT):
                                st = c4_pool.tile([P, W], o_dt, tag="c4")
                                nc.scalar.add(st[:], psums[t][:], bias_ap)
                                rows = S if t < NT - 1 else H - S * (NT - 1)
                                nc.sync.dma_start(
                                    out=out_dram[o, S * t : S * t + rows, :],
                                    in_=st[HALO : HALO + rows, :],
                                )
                    # edge-zero the new planes (reference 'same' zero padding)
                    if c < 3 and not (c == 2 and (conv4_exact or dr4)):
                        outs = (
                            ["c1_0", "c1_1"]
                            if c == 0
                            else (
                                [f"c2_{i}" for i in range(4)]
                                if c == 1
                                else [f"c3_{i}" for i in range(8)]
                            )
                        )
                        for nm in outs:
                            nc.gpsimd.memset(msafe(planes[nm][0:HALO, 0:WPAD]), 0.0)
                            nc.gpsimd.memset(
                                msafe(
                                    planes[nm][
                                        p_hi_last:P, (NT - 1) * WPAD : NT * WPAD
                                    ]
                                ),
                                0.0,
                            )
                    # early out-channel DMAs + exact-layout copies
                    if (conv4_exact or dr4) and c < 2:
                        new_pl = ["c1_0", "c1_1"] if c == 0 else [
                            f"c2_{i}" for i in range(4)
                        ]
                        for nm in new_pl:
                            emit_chans(nm, dedup=dr4)
                            if conv4_exact:
                                for t in range(NT):
                                    rows = S if t < NT - 1 else H - S * (NT - 1)
                                    to_exact(
                                        nm, t, rows,
                                        planes[nm][
                                            :, t * WPAD + 2 : t * WPAD + 514
                                        ],
                                    )
                            else:
                                conv_to_pairs(nm)
                    bias_col += O

                # ---- remaining output channels from stored planes ----
                done_early = (
                    set(nm for nm in PLANE_NAMES)
                    if (conv4_exact or dr4)
                    else set()
                )
                for ch in range(16, 48 if out_mode == 'full' else 16):
                    nm = CH_MAP[ch]
                    if nm in done_early:
                        continue
                    for t in range(NT):
                        rows = S if t < NT - 1 else H - S * (NT - 1)
                        src_ap = planes[nm][
                            HALO : HALO + rows, t * WPAD + 2 : t * WPAD + 514
                        ]
                        if mm == 'bf16':
                            assert out_dt == 'bf16', "bf16 planes need bf16 out"
                            nc.sync.dma_start(
                                out=out_dram[ch, S * t : S * t + rows, :],
                                in_=src_ap,
                            )
                        else:
                            nc.sync.dma_start(
                                out=out_dram[ch, S * t : S * t + rows, :],
                                in_=src_ap.bitcast(F32),
                            )

    nc.compile()
    return nc


_NC_CACHE = None

# validated fast config (HW rel err 4.625e-03 on all 8 cores):
# bf16 planes/matmuls, host-built DMA band tables, bf16 output,
# fp8 DoubleRow conv3+conv4 over paired planes (sq terms kept bf16),
# dedup'd output-channel DMAs via DRAM->DRAM replication.
KCFG = dict(mm='bf16', band_src='dma', out_dt='bf16', dr4=True, dr3=True)


def _get_nc():
    global _NC_CACHE
    if _NC_CACHE is None:
        _NC_CACHE = build_nc(**KCFG)
    return _NC_CACHE


def _in_maps(inputs, n_cores, band_src):
    wtab, ident, bias, bands, c3b = _host_tables(inputs)
    feat = inputs["feature_in"].astype(np.float32)  # [8,1,512,512]
    maps = []
    for b in range(n_cores):
        m = {"p": feat[b, 0], "ident": ident, "wtab": wtab, "bias": bias}
        if band_src == 'dma':
            m["bands"] = bands
            m["c3b"] = c3b
        maps.append(m)
    return maps


def _run(inputs, trace=False):
    inputs = {k: np.asarray(v) for k, v in inputs.items()}
    nc = _get_nc()
    n_cores = inputs["feature_in"].shape[0]
    in_maps = _in_maps(inputs, n_cores, KCFG["band_src"])
    res = bass_utils.run_bass_kernel_spmd(
        nc, in_maps, core_ids=list(range(n_cores)), trace=trace
    )
    out = np.stack([res.results[b]["out"] for b in range(n_cores)], axis=0)
    return out.astype(np.float32), res


def kernel(**inputs):
    return _run(inputs, trace=False)[0]



# revision 69
# speedup vs baseline: 1.0370x; 1.0370x over previous
"""Trainium2 Bass kernel for nn_Encoder_P: unwrap-diff-square front-end + 4 dilated
convs with dense concatenation, fused end-to-end on-chip.

Strategy (pure data parallel, 1 batch sample per NeuronCore, 8 cores):
  - The unwrap/diff/pad chain collapses: cumsum cancels in the diff, so
    sq[h] = wrap(p[h] - p[h-1])^2 (row 0 = 0), wrap(v) = v - 2*pi*k with
    k = (v>=pi) + (v>=3pi) - (v<=-pi) - (v<=-3pi).
  - Duplicate concat channels are folded into effective conv weights
    (conv3: 8->7 input planes, conv4: 20->15).
  - Each conv runs on TensorE as banded matmuls over the H (partition) axis:
    lhsT is a banded [128,128] H-shift matrix; rhs is the input plane tile
    [128 H, 516 Wpad]; PSUM accumulates over (ci, kw). Planes are stored as
    5 overlapping H-tiles (stride 107, halo 12) with zeroed W margins.

Production config (KCFG):
  - bf16 planes + bf16 matmuls; conv1-3 band tables precomputed on host and
    DMA'd in (no per-band DVE build); bf16 DRAM output, upcast on host.
  - conv4 (1x1, 15 planes -> 16 ch) runs as fp8e4m3 DoubleRow matmuls: the
    14 small-magnitude planes are DVE-converted into 7 halo-grid pair
    tensors ([K,2,N] APs, K-doubled contraction, ~0.7x stream cycles per
    the HW DoubleRow rate); sq (magnitude ~pi^2) stays bf16 via one extra
    plain matmul per (o, tile). Matmul stream count: 1980 -> 1420, with
    560 of those at DoubleRow rate.
  - conv3 (2x2 dil3) and conv2 (3x3 dil2) likewise: c2/c1 pairs via fp8
    DoubleRow (pair-bands host-built), sq terms bf16. 1120 total matmul
    streams (860 at DoubleRow rate) vs 1980 in the f32r baseline.
  - Output channels 16-47 stream early (right after their source plane is
    final); duplicated channels are written once from SBUF then replicated
    DRAM->DRAM (1 dma_start each); conv3/conv4 stores batch tiles 0-3 into
    a single 3D-AP dma_start — HWDGE descriptor-gen serial time drops from
    ~224us (naive) to ~72us, well under the TensorE stream time.
"""

import numpy as np

import concourse.bacc as bacc
import concourse.bass as bass
import concourse.mybir as mybir
import concourse.tile as tile
from concourse import bass_utils

F32 = mybir.dt.float32
MM_DT = mybir.dt.float32r  # full-rate fp32 matmul path (1 cyc/row at N>=256)
DEFAULT_MM = "f32r"  # flip to "bf16" only with HW-validated accuracy+speed

H = 512
W = 512
S = 107          # tile stride in rows (chosen so 512-(S*4-HALO) == 96, a legal
                 # compute-op partition start for the bottom edge-zero memset)
HALO = 12        # halo rows above/below each tile
NT = 5           # number of H tiles
WPAD = 516       # 2 zero cols + 512 + 2 zero cols
WA = 528         # fp8 pair-slot stride (16B-aligned for DoubleRow APs)
P = 128
PI = float(np.pi)

# conv specs: (dil, pad_top, pad_left, KH, KW)
CONV_GEOM = [
    (1, 1, 1, 4, 4),   # conv1: 4x4 dil1, 'same' pad (1,2)
    (2, 2, 2, 3, 3),   # conv2: 3x3 dil2, pad (2,2)
    (3, 1, 1, 2, 2),   # conv3: 2x2 dil3, pad (1,2)
    (4, 0, 0, 1, 1),   # conv4: 1x1
]

PLANE_NAMES = (
    ["sq", "c1_0", "c1_1"]
    + [f"c2_{i}" for i in range(4)]
    + [f"c3_{i}" for i in range(8)]
)
CONV_INPUTS = [
    ["sq"],
    ["c1_0", "c1_1", "sq"],
    [f"c2_{i}" for i in range(4)] + ["c1_0", "c1_1", "sq"],
    [f"c3_{i}" for i in range(8)] + [f"c2_{i}" for i in range(4)]
    + ["c1_0", "c1_1", "sq"],
]
CONV_OUT = [2, 4, 8, 16]
DELTAS = [-2, -1, 0, 1, 2]  # identity master shifts

# output channel -> source plane ("c4_o" channels handled separately)
CH_MAP = (
    [f"c4_{i}" for i in range(16)]
    + [f"c3_{i}" for i in range(8)]
    + [f"c2_{i}" for i in range(4)]
    + ["c1_0", "c1_1", "sq", "sq", "c1_0", "c1_1", "sq", "sq"]
    + [f"c2_{i}" for i in range(4)]
    + ["c1_0", "c1_1", "sq", "sq"]
    + ["c1_0", "c1_1", "sq", "sq"]
)

NSCAL = sum(
    CONV_OUT[c] * len(CONV_INPUTS[c]) * CONV_GEOM[c][3] * CONV_GEOM[c][4]
    for c in range(4)
)  # 604


def _fold_weights(w1, w2, w3, w4):
    w3f = np.zeros((8, 7, 2, 2), np.float32)
    w3f[:, :6] = w3[:, :6]
    w3f[:, 6] = w3[:, 6] + w3[:, 7]
    w4f = np.zeros((16, 15, 1, 1), np.float32)
    w4f[:, :12] = w4[:, :12]
    w4f[:, 12] = w4[:, 12] + w4[:, 16]
    w4f[:, 13] = w4[:, 13] + w4[:, 17]
    w4f[:, 14] = w4[:, 14] + w4[:, 15] + w4[:, 18] + w4[:, 19]
    return [w1.astype(np.float32), w2.astype(np.float32), w3f, w4f]


N_BANDS_DMA = 8 + 36 + 112  # conv1-3 (o,ci,kw) bands; conv4 diags built on DVE


def _host_tables(inputs):
    """wtab [128, NSCAL], ident [5*128, 128], bias [128, 30], bands
    [128, (N_BANDS_DMA+1)*128] bf16 host arrays (last slot = identity)."""
    import ml_dtypes

    wf = _fold_weights(inputs["w1"], inputs["w2"], inputs["w3"], inputs["w4"])
    scal = []
    for c in range(4):
        dil, pad_top, _, KH, KW = CONV_GEOM[c]
        for o in range(CONV_OUT[c]):
            for ci in range(len(CONV_INPUTS[c])):
                for kw in range(KW):
                    for kh in range(KH):
                        scal.append(wf[c][o, ci, kh, kw])
    assert len(scal) == NSCAL
    wtab = np.tile(np.asarray(scal, np.float32)[None, :], (P, 1))
    ident = np.concatenate(
        [np.eye(P, dtype=np.float32, k=-d) for d in DELTAS], axis=0
    )
    bias = np.concatenate(
        [inputs["b1"], inputs["b2"], inputs["b3"], inputs["b4"]]
    ).astype(np.float32)
    bias = np.tile(bias[None, :], (P, 1))

    # band tables for conv1-3, partition-major [128, nb*128]
    # trailing slots: identity, half-identity (for the (sq,sq) self-pair)
    bands = np.zeros((P, (N_BANDS_DMA + 2) * P), np.float32)
    j = 0
    for c in range(3):
        dil, pad_top, _, KH, KW = CONV_GEOM[c]
        deltas = [kh * dil - pad_top for kh in range(KH)]
        for o in range(CONV_OUT[c]):
            for ci in range(len(CONV_INPUTS[c])):
                for kw in range(KW):
                    b = np.zeros((P, P), np.float32)
                    for kh, d in enumerate(deltas):
                        b += wf[c][o, ci, kh, kw] * np.eye(P, k=-d)
                    bands[:, j * P : (j + 1) * P] = b
                    j += 1
    assert j == N_BANDS_DMA
    bands[:, j * P : (j + 1) * P] = np.eye(P)
    bands[:, (j + 1) * P : (j + 2) * P] = 0.5 * np.eye(P)
    bands = bands.astype(ml_dtypes.bfloat16)

    # fp8 DoubleRow pair-bands: conv3 (o, pair, kw, slot) in slots 0-95,
    # conv2 (o, kw, slot) with the (c1_0, c1_1) pair in slots 96-119
    ci3 = {"c2_0": 0, "c2_1": 1, "c2_2": 2, "c2_3": 3, "c1_0": 4, "c1_1": 5}
    d3 = [kh * 3 - 1 for kh in range(2)]  # conv3 deltas
    c3b = np.zeros((P, (96 + 24) * P), np.float32)
    for o in range(8):
        for pj, pr in enumerate(PAIRS[4:7]):
            for kw in range(2):
                for i in (0, 1):
                    b = np.zeros((P, P), np.float32)
                    for kh, d in enumerate(d3):
                        b += wf[2][o, ci3[pr[i]], kh, kw] * np.eye(P, k=-d)
                    k = (((o * 3 + pj) * 2 + kw) * 2 + i) * P
                    c3b[:, k : k + P] = b
    d2 = [kh * 2 - 2 for kh in range(3)]  # conv2 deltas
    for o in range(4):
        for kw in range(3):
            for i in (0, 1):
                b = np.zeros((P, P), np.float32)
                for kh, d in enumerate(d2):
                    b += wf[1][o, i, kh, kw] * np.eye(P, k=-d)
                k = (96 + (o * 3 + kw) * 2 + i) * P
                c3b[:, k : k + P] = b
    c3b = c3b.astype(ml_dtypes.float8_e4m3)
    return wtab, ident, bias, bands, c3b


# conv4 DoubleRow plane pairs; sq stays bf16 (its magnitude ~pi^2 makes
# fp8 quantization the dominant output error)
PAIRS = [
    ("c3_0", "c3_1"), ("c3_2", "c3_3"), ("c3_4", "c3_5"), ("c3_6", "c3_7"),
    ("c2_0", "c2_1"), ("c2_2", "c2_3"), ("c1_0", "c1_1"),
]
# plane name -> column in folded w4f
W4COL = {f"c3_{i}": i for i in range(8)}
W4COL.update({f"c2_{i}": 8 + i for i in range(4)})
W4COL.update({"c1_0": 12, "c1_1": 13, "sq": 14})
PAIR_OF = {}
for _j, (_a, _b) in enumerate(PAIRS):
    PAIR_OF.setdefault(_a, []).append((_j, 0))
    PAIR_OF.setdefault(_b, []).append((_j, 1))


def build_nc(loop_k=1, out_mode='full', skip_bands=False, mm='f32r',
             col_split=1, n_convs=4, band_src='dve', out_dt='f32',
             conv4_exact=False, dr4=False, dr3=False):
    assert band_src in ('dve', 'dma')
    if band_src == 'dma':
        assert mm == 'bf16', "dma band tables are bf16"
    if conv4_exact:
        assert mm == 'bf16' and out_dt == 'bf16' and n_convs == 4
    if dr4:
        assert band_src == 'dma' and out_dt == 'bf16' and n_convs == 4
        assert not conv4_exact
    if dr3:
        assert dr4, "dr3 reuses dr4's fp8 pair tensors"
    nc = bacc.Bacc("TRN2", target_bir_lowering=False, debug=False)
    mm_dt = mybir.dt.bfloat16 if mm == 'bf16' else MM_DT
    o_dt = mybir.dt.bfloat16 if out_dt == 'bf16' else F32

    def msafe(ap):
        # memset target: walrus rejects float32r memsets; bitcast those to f32
        return ap.bitcast(F32) if mm != 'bf16' else ap

    p_dram = nc.dram_tensor("p", [H, W], F32, kind="ExternalInput")
    ident_dram = nc.dram_tensor("ident", [5 * P, P], F32, kind="ExternalInput")
    wtab_dram = nc.dram_tensor("wtab", [P, NSCAL], F32, kind="ExternalInput")
    bias_dram = nc.dram_tensor("bias", [P, 30], F32, kind="ExternalInput")
    bands_dram = (
        nc.dram_tensor(
            "bands", [P, (N_BANDS_DMA + 2) * P], mybir.dt.bfloat16,
            kind="ExternalInput",
        )
        if band_src == 'dma'
        else None
    )
    out_dram = nc.dram_tensor("out", [48, H, W], o_dt, kind="ExternalOutput")

    FP8 = mybir.dt.float8e4
    # halo-grid planes; with conv4_exact/dr4, c3 skips the halo layout
    halo_names = [
        nm for nm in PLANE_NAMES
        if not ((conv4_exact or dr4) and nm.startswith("c3_"))
    ]
    planes = {
        nm: nc.alloc_sbuf_tensor(f"pl_{nm}", [P, NT * WPAD], mm_dt)
        for nm in halo_names
    }
    # exact-128-row tiles (4 per plane, no W margins) for conv4 inputs
    xnames = CONV_INPUTS[3] if conv4_exact else []
    xplanes = {
        nm: nc.alloc_sbuf_tensor(f"xp_{nm}", [P, 4 * W], mm_dt)
        for nm in xnames
    }
    # fp8 halo-grid pair-planes for DoubleRow conv4: dim1 = (tile, slot),
    # dim2 = WA cols (16B-aligned pair stride; data at [2, 514))
    fp8_pairs = (
        [
            nc.alloc_sbuf_tensor(f"f8p_{j}", [P, NT * 2, WA], FP8)
            for j in range(len(PAIRS))
        ]
        if dr4
        else None
    )
    # DoubleRow pair-band table: (o, j) -> [128, 2, 128] diag pair
    c4b_sb = (
        nc.alloc_sbuf_tensor("c4b_sb", [P, 16 * len(PAIRS) * 2 * P], FP8)
        if dr4
        else None
    )
    # bf16 diag bands for conv4's sq term
    sqb_sb = (
        nc.alloc_sbuf_tensor("sqb_sb", [P, 16 * P], mybir.dt.bfloat16)
        if dr4
        else None
    )
    # fp8 DoubleRow pair-bands for conv3 (host-built, DMA'd)
    c3b_dram = (
        nc.dram_tensor("c3b", [P, 120 * P], FP8, kind="ExternalInput")
        if dr3
        else None
    )
    c3b_sb = (
        nc.alloc_sbuf_tensor("c3b_sb", [P, 120 * P], FP8) if dr3 else None
    )
    ident_sb = nc.alloc_sbuf_tensor("ident_sb", [P, 5 * P], F32)
    wtab_sb = nc.alloc_sbuf_tensor("wtab_sb", [P, NSCAL], F32)
    bias_sb = nc.alloc_sbuf_tensor("bias_sb", [P, 30], F32)
    bands_sb = (
        nc.alloc_sbuf_tensor(
            "bands_sb", [P, (N_BANDS_DMA + 2) * P], mybir.dt.bfloat16
        )
        if band_src == 'dma'
        else None
    )

    def pslice(nm, t, c0, c1):
        return planes[nm][:, t * WPAD + c0 : t * WPAD + c1]

    with tile.TileContext(nc) as tc:
        with (
            tc.tile_pool(name="io", bufs=3) as io_pool,
            tc.tile_pool(name="front", bufs=1 if (dr3 or dr4) else 2) as fr_pool,
            tc.tile_pool(name="bands", bufs=12) as band_pool,
            tc.tile_pool(name="psum", bufs=8, space="PSUM") as psum_pool,
            tc.tile_pool(name="c4st", bufs=3) as c4_pool,
            tc.tile_pool(name="f8", bufs=3) as f8_pool,
        ):
            for _it in range(loop_k):
                # ---- parameter loads ----
                if band_src == 'dve' or skip_bands:
                    for j in range(5):
                        nc.sync.dma_start(
                            out=ident_sb[:, j * P : (j + 1) * P],
                            in_=ident_dram[j * P : (j + 1) * P, :],
                        )
                if band_src == 'dma':
                    nc.sync.dma_start(out=bands_sb[:], in_=bands_dram[:])
                if dr3:
                    nc.sync.dma_start(out=c3b_sb[:], in_=c3b_dram[:])
                    # zero pair tensors so W margins read as reference padding
                    for j_ in range(len(PAIRS)):
                        for s_ in range(NT * 2):
                            nc.gpsimd.memset(
                                fp8_pairs[j_][:, s_, :].bitcast(F32), 0.0
                            )
                nc.sync.dma_start(out=wtab_sb[:], in_=wtab_dram[:])
                nc.sync.dma_start(out=bias_sb[:], in_=bias_dram[:])

                # ---- zero W margins of all planes (written once) ----
                for nm in halo_names:
                    for t in range(NT):
                        nc.gpsimd.memset(msafe(pslice(nm, t, 0, 2)), 0.0)
                        nc.gpsimd.memset(msafe(pslice(nm, t, 514, 516)), 0.0)

                def emit_chans(nm, dedup=False):
                    """DMA output channels sourced from plane nm. With dedup,
                    write the first channel from SBUF and replicate the rest
                    via DRAM->DRAM copies (1 dma_start each vs NT)."""
                    if out_mode != 'full':
                        return
                    chans = [ch for ch in range(16, 48) if CH_MAP[ch] == nm]
                    for ci_, ch in enumerate(chans):
                        if dedup and ci_ > 0:
                            nc.sync.dma_start(
                                out=out_dram[ch, :, :],
                                in_=out_dram[chans[0], :, :],
                            )
                            continue
                        for t in range(NT):
                            rows = S if t < NT - 1 else H - S * (NT - 1)
                            nc.sync.dma_start(
                                out=out_dram[ch, S * t : S * t + rows, :],
                                in_=planes[nm][
                                    HALO : HALO + rows,
                                    t * WPAD + 2 : t * WPAD + 514,
                                ],
                            )

                def to_exact(nm, t, rows, src_tile):
                    """DMA rows [S*t, S*t+rows) of src_tile (halo-grid tile:
                    partition HALO+r = image row S*t+r, 512-wide cols) into
                    xplanes[nm] exact tiles."""
                    r0 = S * t
                    r1 = r0 + rows
                    e0, e1 = r0 // 128, (r1 - 1) // 128
                    for e in range(e0, e1 + 1):
                        lo = max(r0, 128 * e)
                        hi = min(r1, 128 * (e + 1))
                        nc.sync.dma_start(
                            out=xplanes[nm][lo - 128 * e : hi - 128 * e,
                                            e * W : e * W + W],
                            in_=src_tile[HALO + lo - r0 : HALO + hi - r0, :],
                        )

                def to_pair(nm, t, src_ap):
                    """DVE-convert a halo-grid [128, 512] bf16 AP into the fp8
                    pair slot(s) of plane nm at tile t (same partitions)."""
                    for (pj, pi) in PAIR_OF.get(nm, []):
                        nc.vector.tensor_scalar(
                            fp8_pairs[pj][:, 2 * t + pi, 2:514],
                            src_ap, 1.0, None, mybir.AluOpType.mult,
                        )

                def conv_to_pairs(nm):
                    for t in range(NT):
                        to_pair(
                            nm, t, planes[nm][:, t * WPAD + 2 : t * WPAD + 514]
                        )

                # ---- front-end: sq ----
                # A/B garbage regions are pre-zeroed so the out-of-image rows
                # compute v=0 -> sq=0, which is exactly the reference's zero pad.
                for t in range(NT):
                    p_lo = HALO if t == 0 else 0
                    p_hi = H - (S * (NT - 1) - HALO) if t == NT - 1 else P  # 96 at t=4
                    n = p_hi - p_lo
                    r_lo = S * t - HALO + p_lo
                    A = io_pool.tile([P, W], F32, tag="A")
                    B = io_pool.tile([P, W], F32, tag="B")
                    if t == 0:
                        nc.gpsimd.memset(A[0:32, :], 0.0)
                        nc.gpsimd.memset(B[0:32, :], 0.0)
                    if t == NT - 1:
                        nc.gpsimd.memset(A[96:P, :], 0.0)
                        nc.gpsimd.memset(B[96:P, :], 0.0)
                    nc.sync.dma_start(out=A[p_lo:p_hi, :], in_=p_dram[r_lo : r_lo + n, :])
                    if t == 0:
                        nc.sync.dma_start(
                            out=B[p_lo + 1 : p_hi, :], in_=p_dram[0 : n - 1, :]
                        )
                        nc.sync.dma_start(out=B[p_lo : p_lo + 1, :], in_=p_dram[0:1, :])
                    else:
                        nc.sync.dma_start(
                            out=B[p_lo:p_hi, :], in_=p_dram[r_lo - 1 : r_lo - 1 + n, :]
                        )
                    V = fr_pool.tile([P, W], F32, tag="V")
                    K1 = fr_pool.tile([P, W], F32, tag="K1")
                    K2 = fr_pool.tile([P, W], F32, tag="K2")
                    K3 = fr_pool.tile([P, W], F32, tag="K3")
                    K4 = fr_pool.tile([P, W], F32, tag="K4")
                    ao = mybir.AluOpType
                    nc.vector.tensor_tensor(V[:], A[:], B[:], ao.subtract)
                    nc.vector.tensor_scalar(K1[:], V[:], PI, None, ao.is_ge)
                    nc.vector.tensor_scalar(K2[:], V[:], 3 * PI, None, ao.is_ge)
                    nc.vector.tensor_scalar(K3[:], V[:], -PI, None, ao.is_le)
                    nc.vector.tensor_scalar(K4[:], V[:], -3 * PI, None, ao.is_le)
                    nc.vector.tensor_tensor(K1[:], K1[:], K2[:], ao.add)
                    nc.vector.tensor_tensor(K3[:], K3[:], K4[:], ao.add)
                    nc.vector.tensor_tensor(K1[:], K1[:], K3[:], ao.subtract)
                    nc.vector.scalar_tensor_tensor(
                        V[:], K1[:], -2 * PI, V[:], ao.mult, ao.add
                    )
                    sq_dst = planes["sq"][:, t * WPAD + 2 : t * WPAD + 514]
                    nc.vector.tensor_tensor(sq_dst, V[:], V[:], ao.mult)

                if conv4_exact or dr4:
                    emit_chans("sq", dedup=dr4)
                    if conv4_exact:
                        for t in range(NT):
                            rows = S if t < NT - 1 else H - S * (NT - 1)
                            to_exact(
                                "sq", t, rows,
                                planes["sq"][:, t * WPAD + 2 : t * WPAD + 514],
                            )

                if dr4:
                    # DoubleRow pair-band table: diag(w) pairs, built on DVE
                    base4 = NSCAL - 240
                    NPJ = len(PAIRS)
                    ident_ap = bands_sb[
                        :, N_BANDS_DMA * P : (N_BANDS_DMA + 1) * P
                    ]
                    for o4 in range(16):
                        for pj, pr in enumerate(PAIRS):
                            for pi in (0, 1):
                                col = base4 + o4 * 15 + W4COL[pr[pi]]
                                k = ((o4 * NPJ + pj) * 2 + pi) * P
                                nc.vector.tensor_scalar(
                                    c4b_sb[:, k : k + P],
                                    ident_ap,
                                    wtab_sb[:, col : col + 1],
                                    None,
                                    mybir.AluOpType.mult,
                                )
                        colq = base4 + o4 * 15 + 14
                        nc.vector.tensor_scalar(
                            sqb_sb[:, o4 * P : (o4 + 1) * P],
                            ident_ap,
                            wtab_sb[:, colq : colq + 1],
                            None,
                            mybir.AluOpType.mult,
                        )

                # ---- convs ----
                jcol = 0
                bias_col = 0
                bidx = 0
                p_hi_last = H - (S * (NT - 1) - HALO)  # 108
                for c in range(n_convs):
                    dil, pad_top, pad_left, KH, KW = CONV_GEOM[c]
                    in_names = CONV_INPUTS[c]
                    O = CONV_OUT[c]
                    deltas = [kh * dil - pad_top for kh in range(KH)]
                    if c == 1 and dr3:
                        DR = mybir.MatmulPerfMode.DoubleRow
                        for o in range(O):
                            psums = [
                                psum_pool.tile(
                                    [P, W], F32, tag="ps", name=f"ps2_{o}_{t}"
                                )
                                for t in range(NT)
                            ]
                            for kw in range(3):
                                k = (96 + (o * 3 + kw) * 2) * P
                                lhsT = c3b_sb[:, k : k + 2 * P].rearrange(
                                    "p (two m) -> p two m", two=2
                                )
                                coff = 2 + kw * 2 - 2
                                for t in range(NT):
                                    rhs = fp8_pairs[6][
                                        :, 2 * t : 2 * t + 2,
                                        coff : coff + W,
                                    ]
                                    nc.tensor.matmul(
                                        psums[t], lhsT, rhs,
                                        start=(kw == 0),
                                        stop=False,
                                        perf_mode=DR,
                                    )
                            for kw in range(3):
                                bidx2 = 8 + (o * 3 + 2) * 3 + kw
                                band = bands_sb[
                                    :, bidx2 * P : (bidx2 + 1) * P
                                ]
                                coff = 2 + kw * 2 - 2
                                for t in range(NT):
                                    nc.tensor.matmul(
                                        psums[t],
                                        band,
                                        planes["sq"][
                                            :,
                                            t * WPAD + coff : t * WPAD
                                            + coff + W,
                                        ],
                                        start=False,
                                        stop=(kw == 2),
                                    )
                            bias_ap = bias_sb[:, bias_col + o : bias_col + o + 1]
                            for t in range(NT):
                                nc.scalar.add(
                                    pslice(f"c2_{o}", t, 2, 514),
                                    psums[t][:], bias_ap,
                                )
                        for nm in [f"c2_{i}" for i in range(4)]:
                            nc.gpsimd.memset(msafe(planes[nm][0:HALO, 0:WPAD]), 0.0)
                            nc.gpsimd.memset(
                                msafe(
                                    planes[nm][
                                        p_hi_last:P, (NT - 1) * WPAD : NT * WPAD
                                    ]
                                ),
                                0.0,
                            )
                            emit_chans(nm, dedup=True)
                            conv_to_pairs(nm)
                        bias_col += O
                        jcol += 108
                        continue

                    if c == 2 and dr3:
                        DR = mybir.MatmulPerfMode.DoubleRow
                        c3base = 8 + 36
                        for o in range(O):
                            psums = [
                                psum_pool.tile(
                                    [P, W], F32, tag="ps", name=f"ps3_{o}_{t}"
                                )
                                for t in range(NT)
                            ]
                            for pj3 in range(3):
                                for kw in range(2):
                                    k = (((o * 3 + pj3) * 2 + kw) * 2) * P
                                    lhsT = c3b_sb[:, k : k + 2 * P].rearrange(
                                        "p (two m) -> p two m", two=2
                                    )
                                    coff = 2 + kw * 3 - 1
                                    for t in range(NT):
                                        rhs = fp8_pairs[4 + pj3][
                                            :, 2 * t : 2 * t + 2,
                                            coff : coff + W,
                                        ]
                                        nc.tensor.matmul(
                                            psums[t], lhsT, rhs,
                                            start=(pj3 == 0 and kw == 0),
                                            stop=False,
                                            perf_mode=DR,
                                        )
                            for kw in range(2):
                                bidx3 = c3base + (o * 7 + 6) * 2 + kw
                                band = bands_sb[
                                    :, bidx3 * P : (bidx3 + 1) * P
                                ]
                                coff = 2 + kw * 3 - 1
                                for t in range(NT):
                                    nc.tensor.matmul(
                                        psums[t],
                                        band,
                                        planes["sq"][
                                            :,
                                            t * WPAD + coff : t * WPAD
                                            + coff + W,
                                        ],
                                        start=False,
                                        stop=(kw == 1),
                                    )
                            bias_ap = bias_sb[:, bias_col + o : bias_col + o + 1]
                            st4 = c4_pool.tile([P, 4 * W], o_dt, tag="c4w")
                            for t in range(4):
                                nc.scalar.add(
                                    st4[:, t * W : (t + 1) * W],
                                    psums[t][:], bias_ap,
                                )
                                to_pair(
                                    f"c3_{o}", t, st4[:, t * W : (t + 1) * W]
                                )
                            nc.sync.dma_start(
                                out=out_dram[16 + o, 0 : 4 * S, :].rearrange(
                                    "(t r) w -> r t w", t=4
                                ),
                                in_=st4[HALO : HALO + S, :].rearrange(
                                    "p (t w) -> p t w", t=4
                                ),
                            )
                            st = c4_pool.tile([P, W], o_dt, tag="c4")
                            nc.scalar.add(st[:], psums[4][:], bias_ap)
                            nc.sync.dma_start(
                                out=out_dram[16 + o, 4 * S : H, :],
                                in_=st[HALO : HALO + H - 4 * S, :],
                            )
                            to_pair(f"c3_{o}", 4, st[:])
                        bias_col += O
                        jcol += 224
                        continue

                    if c == 3 and dr4:
                        DR = mybir.MatmulPerfMode.DoubleRow
                        for o in range(O):
                            psums = [
                                psum_pool.tile(
                                    [P, W], F32, tag="ps", name=f"ps4_{o}_{t}"
                                )
                                for t in range(NT)
                            ]
                            for pj in range(len(PAIRS)):
                                k = (o * len(PAIRS) + pj) * 2 * P
                                lhsT = c4b_sb[:, k : k + 2 * P].rearrange(
                                    "p (two m) -> p two m", two=2
                                )
                                for t in range(NT):
                                    rhs = fp8_pairs[pj][:, 2 * t : 2 * t + 2, 2:514]
                                    nc.tensor.matmul(
                                        psums[t], lhsT, rhs,
                                        start=(pj == 0),
                                        stop=False,
                                        perf_mode=DR,
                                    )
                            for t in range(NT):
                                nc.tensor.matmul(
                                    psums[t],
                                    sqb_sb[:, o * P : (o + 1) * P],
                                    planes["sq"][
                                        :, t * WPAD + 2 : t * WPAD + 514
                                    ],
                                    start=False,
                                    stop=True,
                                )
                            bias_ap = bias_sb[:, bias_col + o : bias_col + o + 1]
                            st4 = c4_pool.tile([P, 4 * W], o_dt, tag="c4w")
                            for t in range(4):
                                nc.scalar.add(
                                    st4[:, t * W : (t + 1) * W],
                                    psums[t][:], bias_ap,
                                )
                            nc.sync.dma_start(
                                out=out_dram[o, 0 : 4 * S, :].rearrange(
                                    "(t r) w -> r t w", t=4
                                ),
                                in_=st4[HALO : HALO + S, :].rearrange(
                                    "p (t w) -> p t w", t=4
                                ),
                            )
                            st = c4_pool.tile([P, W], o_dt, tag="c4")
                            nc.scalar.add(st[:], psums[4][:], bias_ap)
                            nc.sync.dma_start(
                                out=out_dram[o, 4 * S : H, :],
                                in_=st[HALO : HALO + H - 4 * S, :],
                            )
                        jcol += 240
                        bias_col += O
                        continue
                    x4 = conv4_exact and c == 3
                    NTC = 4 if x4 else NT
                    for o in range(O):
                        psums = [
                            psum_pool.tile([P, W], F32, tag="ps", name=f"ps_{c}_{o}_{t}")
                            for t in range(NTC)
                        ]
                        for ci, nm in enumerate(in_names):
                            for kw in range(KW):
                                use_dma_band = band_src == 'dma' and c < 3
                                if skip_bands:
                                    band = None
                                    jcol += len(deltas)
                                    if use_dma_band:
                                        bidx += 1
                                elif use_dma_band:
                                    band = bands_sb[:, bidx * P : (bidx + 1) * P]
                                    bidx += 1
                                    jcol += len(deltas)
                                elif band_src == 'dma':
                                    # conv4: diagonal band = w * I, one DVE op
                                    band = band_pool.tile([P, P], mm_dt, tag="band")
                                    w_ap = wtab_sb[:, jcol : jcol + 1]
                                    jcol += len(deltas)
                                    nc.vector.tensor_scalar(
                                        band[:],
                                        bands_sb[
                                            :, N_BANDS_DMA * P : (N_BANDS_DMA + 1) * P
                                        ],
                                        w_ap,
                                        None,
                                        mybir.AluOpType.mult,
                                    )
                                else:
                                    band = band_pool.tile([P, P], mm_dt, tag="band")
                                    for i, d in enumerate(deltas):
                                        w_ap = wtab_sb[:, jcol : jcol + 1]
                                        jcol += 1
                                        src = ident_sb[
                                            :, (d + 2) * P : (d + 3) * P
                                        ]
                                        ao = mybir.AluOpType
                                        if i == 0:
                                            nc.vector.tensor_scalar(
                                                band[:], src, w_ap, None, ao.mult
                                            )
                                        else:
                                            nc.vector.scalar_tensor_tensor(
                                                band[:], src, w_ap, band[:],
                                                ao.mult, ao.add
                                            )
                                coff = 2 + kw * dil - pad_left
                                first = ci == 0 and kw == 0
                                last = ci == len(in_names) - 1 and kw == KW - 1
                                for t in range(NTC):
                                    rhs = (
                                        xplanes[nm][:, t * W : (t + 1) * W]
                                        if x4
                                        else planes[nm][
                                            :, t * WPAD + coff : t * WPAD + coff + W
                                        ]
                                    )
                                    if skip_bands:
                                        lhsT = (
                                            ident_sb[:, 2 * P : 3 * P].bitcast(mm_dt)
                                            if mm != "bf16"
                                            else ident_sb[:, 2 * P : 3 * P]
                                        )
                                    elif use_dma_band:
                                        lhsT = band
                                    else:
                                        lhsT = band[:]
                                    if col_split == 1:
                                        nc.tensor.matmul(
                                            psums[t], lhsT, rhs,
                                            start=first, stop=last,
                                        )
                                    else:
                                        cw = P // col_split
                                        for j in range(col_split):
                                            nc.tensor.matmul(
                                                psums[t][j * cw : (j + 1) * cw, :],
                                                lhsT[:, j * cw : (j + 1) * cw],
                                                rhs,
                                                start=first,
                                                stop=last,
                                                tile_position=(0, j * cw),
                                                skip_group_check=True,
                                            )
                        bias_ap = bias_sb[:, bias_col + o : bias_col + o + 1]
                        if c == 2 and (conv4_exact or dr4):
                            # conv3: evac to scratch; c3 out channel + exact repack
                            for t in range(NT):
                                st = c4_pool.tile([P, W], o_dt, tag="c4")
                                nc.scalar.add(st[:], psums[t][:], bias_ap)
                                rows = S if t < NT - 1 else H - S * (NT - 1)
                                nc.sync.dma_start(
                                    out=out_dram[16 + o, S * t : S * t + rows, :],
                                    in_=st[HALO : HALO + rows, :],
                                )
                                if conv4_exact:
                                    to_exact(f"c3_{o}", t, rows, st[:, :])
                                else:
                                    to_pair(f"c3_{o}", t, st[:])
                        elif c < 3:
                            out_nm = (
                                ["c1_0", "c1_1"][o]
                                if c == 0
                                else (f"c2_{o}" if c == 1 else f"c3_{o}")
                            )
                            for t in range(NT):
                                nc.scalar.add(
                                    pslice(out_nm, t, 2, 514), psums[t][:], bias_ap
                                )
                        elif x4:
                            for e in range(4):
                                st = c4_pool.tile([P, W], o_dt, tag="c4")
                                nc.scalar.add(st[:], psums[e][:], bias_ap)
                                nc.sync.dma_start(
                                    out=out_dram[o, 128 * e : 128 * (e + 1), :],
                                    in_=st[:],
                                )
                        else:
                            for t in range(NT):
                                st = c4_pool.tile([P, W], o_dt, tag="c4")
                                nc.scalar.add(st[:], psums[t][:], bias_ap)
                                rows = S if t < NT - 1 else H - S * (NT - 1)
                                nc.sync.dma_start(
                                    out=out_dram[o, S * t : S * t + rows, :],
                                    in_=st[HALO : HALO + rows, :],
                                )
                    # edge-zero the new planes (reference 'same' zero padding)
                    if c < 3 and not (c == 2 and (conv4_exact or dr4)):
                        outs = (
                            ["c1_0", "c1_1"]
                            if c == 0
                            else (
                                [f"c2_{i}" for i in range(4)]
                                if c == 1
                                else [f"c3_{i}" for i in range(8)]
                            )
                        )
                        for nm in outs:
                            nc.gpsimd.memset(msafe(planes[nm][0:HALO, 0:WPAD]), 0.0)
                            nc.gpsimd.memset(
                                msafe(
                                    planes[nm][
                                        p_hi_last:P, (NT - 1) * WPAD : NT * WPAD
                                    ]
                                ),
                                0.0,
                            )
                    # early out-channel DMAs + exact-layout copies
                    if (conv4_exact or dr4) and c < 2:
                        new_pl = ["c1_0", "c1_1"] if c == 0 else [
                            f"c2_{i}" for i in range(4)
                        ]
                        for nm in new_pl:
                            emit_chans(nm, dedup=dr4)
                            if conv4_exact:
                                for t in range(NT):
                                    rows = S if t < NT - 1 else H - S * (NT - 1)
                                    to_exact(
                                        nm, t, rows,
                                        planes[nm][
                                            :, t * WPAD + 2 : t * WPAD + 514
                                        ],
                                    )
                            else:
                                conv_to_pairs(nm)
                    bias_col += O

                # ---- remaining output channels from stored planes ----
                done_early = (
                    set(nm for nm in PLANE_NAMES)
                    if (conv4_exact or dr4)
                    else set()
                )
                for ch in range(16, 48 if out_mode == 'full' else 16):
                    nm = CH_MAP[ch]
                    if nm in done_early:
                        continue
                    for t in range(NT):
                        rows = S if t < NT - 1 else H - S * (NT - 1)
                        src_ap = planes[nm][
                            HALO : HALO + rows, t * WPAD + 2 : t * WPAD + 514
                        ]
                        if mm == 'bf16':
                            assert out_dt == 'bf16', "bf16 planes need bf16 out"
                            nc.sync.dma_start(
                                out=out_dram[ch, S * t : S * t + rows, :],
                                in_=src_ap,
                            )
                        else:
                            nc.sync.dma_start(
                                out=out_dram[ch, S * t : S * t + rows, :],
                                in_=src_ap.bitcast(F32),
                            )

    nc.compile()
    return nc


_NC_CACHE = None

# validated fast config (HW rel err 4.625e-03 on all 8 cores):
# bf16 planes/matmuls, host-built DMA band tables, bf16 output,
# fp8 DoubleRow conv3+conv4 over paired planes (sq terms kept bf16),
# dedup'd output-channel DMAs via DRAM->DRAM replication.
KCFG = dict(mm='bf16', band_src='dma', out_dt='bf16', dr4=True, dr3=True)


def _get_nc():
    global _NC_CACHE
    if _NC_CACHE is None:
        _NC_CACHE = build_nc(**KCFG)
    return _NC_CACHE


def _in_maps(inputs, n_cores, band_src):
    wtab, ident, bias, bands, c3b = _host_tables(inputs)
    feat = inputs["feature_in"].astype(np.float32)  # [8,1,512,512]
    maps = []
    for b in range(n_cores):
        m = {"p": feat[b, 0], "ident": ident, "wtab": wtab, "bias": bias}
        if band_src == 'dma':
            m["bands"] = bands
            m["c3b"] = c3b
        maps.append(m)
    return maps


def _run(inputs, trace=False):
    inputs = {k: np.asarray(v) for k, v in inputs.items()}
    nc = _get_nc()
    n_cores = inputs["feature_in"].shape[0]
    in_maps = _in_maps(inputs, n_cores, KCFG["band_src"])
    res = bass_utils.run_bass_kernel_spmd(
        nc, in_maps, core_ids=list(range(n_cores)), trace=trace
    )
    out = np.stack([res.results[b]["out"] for b in range(n_cores)], axis=0)
    return out.astype(np.float32), res


def kernel(**inputs):
    return _run(inputs, trace=False)[0]

